# revision 12
# baseline (speedup 1.0000x reference)
"""Trainium2 Bass kernel for nn_PositionEncoder (gnn_message_passing).

Strategy
--------
Data parallel over batch B=8: core b processes graph b (sharding_hint).

The reference reduces to four big elementwise streaming updates plus one
small output:

    out0 = edge_attention     + expand(node_s, c_na)     [128,128,64]
    out1 = edge_values        + expand(node_s, c_nv)     [128,128,64]
    out2 = e2e_edge_attention + expand(e2e_s,  c_ea)     [256,256,64]
    out3 = e2e_edge_values    + expand(e2e_s,  c_ev)     [256,256,64]
    out4 = expand(bdl_s, c_b)                            [256,64]

where every `expand` is a rank-4 broadcast product
    delta[i,j,d] = sum_k s_k[i,j] * c_k[d]
because:
  * dis_att/dis_val = Linear(rbf(dis)) with rbf giving 3 scalar fields
    r_k[i,j] (plus the bias as a 4th, constant field),
  * cos(arccos(clip(q))) == clip(q), so cbf is a per-element quadratic in
    x = clip(cos*m, -1, 1) and cos_att/cos_val reduce to
    u*alpha[d] + x*beta[d] + x^2*gamma[d]  (u = |m|).

The s_k fields are tiny ([N,N] / [E,E] scalars, ~2% of the bytes) and are
precomputed on host into a packed lhsT layout; the device streams the 40MB
per-core payload once, expanding the rank-4 products on the TensorEngine,
then one VectorEngine add per 2048-column PSUM quad, and streams out.

TensorEngine details: fp32 matmuls run at 1/4 rate, so operands are fp16
with double-fp16 compensation — each rank k is split into (hi, lo) halves
and the K dim packs three product blocks (s_hi*c_hi + s_hi*c_lo +
s_lo*c_hi), recovering ~fp32 accuracy while keeping fp16 throughput
(LDWEIGHTS cost depends on columns, matmul cost on N; K is nearly free).
rhs holds c_k[d] on a (k,jl) block diagonal so one K=96 matmul produces a
[128 rows, 8*64] PSUM block. This keeps the kernel at the HBM roofline
(memory-bound regime).
"""

import numpy as np

import concourse.bacc as bacc
import concourse.mybir as mybir
from concourse.tile import TileContext
from concourse.bass_utils import run_bass_kernel_spmd

# Problem shapes (hardcoded per task instructions).
B, N, E, D = 8, 128, 256, 64
CUTOFF = 5.0
P = 6  # envelope_exponent 5 -> p = 6
EA = -(P + 1) * (P + 2) / 2.0
EB = float(P * (P + 2))
EC = -P * (P + 1) / 2.0
C0 = 0.28209479177387814
C1 = 0.4886025119029199
C2 = 0.31539156525252005

JW = 8            # j's packed per matmul group
KB = 32           # 4 ranks x JW (one product block)
K = 3 * KB        # 96: [s_hi | s_hi | s_lo] x [c_hi | c_lo | c_hi]
NODE_G = N // JW          # 16 groups
E2E_G = E // JW           # 32 groups
NODE_W = N * D            # 8192 free cols per node tile
E2E_W = E * D             # 16384 free cols per e2e row-chunk

# Packed-constant tensor column layout: [K=96, PACKW] fp16
NODE_LHST_C = NODE_G * 128            # 2048
E2E_LHST_C = 2 * E2E_G * 128          # 8192
RHS_C = JW * D                        # 512
OFF_NODE = 0
OFF_E2E = OFF_NODE + NODE_LHST_C
OFF_RNA = OFF_E2E + E2E_LHST_C
OFF_RNV = OFF_RNA + RHS_C
OFF_REA = OFF_RNV + RHS_C
OFF_REV = OFF_REA + RHS_C
OFF_BDL = OFF_REV + RHS_C             # bdl lhsT: rows 0..11, E cols
OFF_BRHS = OFF_BDL + E                # bdl rhs: rows 0..11, D cols
PACKW = OFF_BRHS + D

QW = 2048          # PSUM quad width (4 banks)
GPQ = QW // RHS_C  # matmul groups per quad = 4
CW = 4096          # DMA chunk width (2 quads)

_CACHED_NC = None
_LAST_RESULTS = None


def _build_nc():
    nc = bacc.Bacc(None, target_bir_lowering=False)
    f32 = mybir.dt.float32
    f16 = mybir.dt.float16

    ea = nc.dram_tensor("ea", [N, NODE_W], f32, kind="ExternalInput")
    ev = nc.dram_tensor("ev", [N, NODE_W], f32, kind="ExternalInput")
    e2a = nc.dram_tensor("e2a", [E, E2E_W], f32, kind="ExternalInput")
    e2v = nc.dram_tensor("e2v", [E, E2E_W], f32, kind="ExternalInput")
    pk = nc.dram_tensor("pk", [K, PACKW], f16, kind="ExternalInput")

    oa = nc.dram_tensor("oa", [N, NODE_W], f32, kind="ExternalOutput")
    ov = nc.dram_tensor("ov", [N, NODE_W], f32, kind="ExternalOutput")
    o2a = nc.dram_tensor("o2a", [E, E2E_W], f32, kind="ExternalOutput")
    o2v = nc.dram_tensor("o2v", [E, E2E_W], f32, kind="ExternalOutput")
    obdl = nc.dram_tensor("obdl", [E, D], f32, kind="ExternalOutput")

    with TileContext(nc) as tc:
        with (
            tc.tile_pool(name="const", bufs=1) as cpool,
            tc.tile_pool(name="sbuf", bufs=10) as pool,
            tc.tile_pool(name="psum", bufs=2, space="PSUM") as psum_pool,
            tc.tile_pool(name="bdl", bufs=2) as bdl_pool,
        ):
            pk_t = cpool.tile([K, PACKW], f16)
            nc.sync.dma_start(out=pk_t[:], in_=pk[:])

            # bdl output: 2 chunks of 128 edges
            for c in range(2):
                ps = psum_pool.tile([128, D], f32, tag="ps")
                nc.tensor.matmul(
                    ps[:],
                    pk_t[0:12, OFF_BDL + c * 128:OFF_BDL + (c + 1) * 128],
                    pk_t[0:12, OFF_BRHS:OFF_BRHS + D],
                    start=True, stop=True,
                )
                st = bdl_pool.tile([128, D], f32)
                nc.scalar.copy(out=st[:], in_=ps[:])
                nc.scalar.dma_start(out=obdl[c * 128:(c + 1) * 128, :], in_=st[:])

            # (dram_in, dram_out, rhs column base, lhsT col base, row chunks)
            work = [
                (ea, oa, OFF_RNA, OFF_NODE, 1, NODE_W),
                (ev, ov, OFF_RNV, OFF_NODE, 1, NODE_W),
                (e2a, o2a, OFF_REA, OFF_E2E, 2, E2E_W),
                (e2v, o2v, OFF_REV, OFF_E2E, 2, E2E_W),
            ]
            for src, dst, rhs_off, lhs_off, nchunks, width in work:
                rhs_ap = pk_t[:, rhs_off:rhs_off + RHS_C]
                for ic in range(nchunks):
                    r0 = ic * 128
                    # lhsT columns for this row-chunk start at lhs_off + ic*G*128
                    gbase = lhs_off + ic * (width // RHS_C) * 128
                    for cc in range(width // CW):
                        t = pool.tile([128, CW], f32)
                        nc.sync.dma_start(
                            out=t[:],
                            in_=src[r0:r0 + 128, cc * CW:(cc + 1) * CW],
                        )
                        for half in range(CW // QW):
                            ps = psum_pool.tile([128, QW], f32, tag="ps")
                            for q in range(GPQ):
                                g = (cc * (CW // QW) + half) * GPQ + q
                                nc.tensor.matmul(
                                    ps[:, q * RHS_C:(q + 1) * RHS_C],
                                    pk_t[:, gbase + g * 128:gbase + (g + 1) * 128],
                                    rhs_ap,
                                    start=True, stop=True,
                                )
                            h0 = half * QW
                            nc.vector.tensor_add(
                                out=t[:, h0:h0 + QW],
                                in0=t[:, h0:h0 + QW],
                                in1=ps[:],
                            )
                        # stores on the ACT HWDGE ring so loads (sync
                        # ring) and stores drain concurrently
                        nc.scalar.dma_start(
                            out=dst[r0:r0 + 128, cc * CW:(cc + 1) * CW],
                            in_=t[:],
                        )
    nc.finalize()
    return nc


def _envelope_rbf(x, freq):
    """r_k = envelope(x) * sin(freq_k * x); freq [3]."""
    inv = 1.0 / x
    x4 = (x * x) * (x * x)
    xp0 = x4 * x  # x^5
    env = inv + EA * xp0 + EB * xp0 * x + EC * xp0 * x * x
    env = np.where(x < 1.0, env, 0.0)
    return env[..., None] * np.sin(freq * x[..., None])


def _hi_lo(a):
    """Split float64 array into fp16 hi + fp16 lo with a ~= hi + lo."""
    hi = a.astype(np.float16)
    lo = (a - hi.astype(np.float64)).astype(np.float16)
    return hi, lo


def _host_factors(pos, freq, src, dst):
    """Packed lhsT factor tensors for all batches (fp16 hi/lo blocks)."""
    # --- node pairwise rbf factors ---
    diff = pos[:, :, None, :] - pos[:, None, :, :]          # [B,N,N,3]
    dis = np.sqrt((diff * diff).sum(-1))                    # [B,N,N]
    x = (dis + 1e-6) / CUTOFF
    r = _envelope_rbf(x, freq)                              # [B,N,N,3]
    # block[b, k*8+jl, g*128+n1] = s_k[b, n1, g*8+jl]
    A = r.transpose(0, 3, 2, 1)                             # [B,3,n2,n1]
    A = A.reshape(B, 3, NODE_G, JW, N).transpose(0, 1, 3, 2, 4)
    A = A.reshape(B, 24, NODE_G * N)
    node_blk = np.concatenate(
        [A, np.ones((B, 8, NODE_G * N))], axis=1)           # [B,32,2048] f64

    # --- edge vectors ---
    vec = (np.take_along_axis(pos, dst[..., None], axis=1)
           - np.take_along_axis(pos, src[..., None], axis=1))  # [B,E,3]
    vn = np.sqrt((vec * vec).sum(-1))                       # [B,E]
    rb = _envelope_rbf(vn / CUTOFF, freq)                   # [B,E,3]
    bdl_blk = np.concatenate(
        [rb.transpose(0, 2, 1), np.ones((B, 1, E))], axis=1)  # [B,4,E] f64

    # --- e2e cos factors ---
    g = np.einsum('bid,bjd->bij', vec, vec)                 # [B,E,E]
    cos = g / (vn[:, None, :] + 1e-6) / (vn[:, :, None] + 1e-6)
    si, sj = src[:, :, None], src[:, None, :]
    di, dj = dst[:, :, None], dst[:, None, :]
    m = np.zeros((B, E, E), np.int64)
    m = np.where((si == sj) & (di != dj), 1, m)
    m = np.where((di == dj) & (si != sj), 1, m)
    m = np.where(si == dj, -1, m)
    m = np.where(di == sj, -1, m)
    u = np.abs(m).astype(np.float64)
    xh = np.clip(cos * m, -1.0, 1.0)
    s = np.stack([u, xh, xh * xh, np.zeros_like(u)], axis=1)  # [B,4,i,j]
    # block[b, k*8+jl, (ic*32+g)*128+il] = s_k[b, ic*128+il, g*8+jl];
    # all s_k are symmetric in (i,j) -> use [b, k, j, i] directly.
    S = s.transpose(0, 1, 3, 2)
    S = S.reshape(B, 4, E2E_G, JW, E).transpose(0, 1, 3, 2, 4)  # [B,4,jl,g,i]
    S = S.reshape(B, 4, JW, E2E_G, 2, 128).transpose(0, 1, 2, 4, 3, 5)
    e2e_blk = S.reshape(B, KB, E2E_LHST_C)                  # f64
    return node_blk, e2e_blk, bdl_blk


def _lhsT_stack(blk):
    """[.., KB, C] f64 -> [.., 3*KB, C] fp16 as [hi; hi; lo]."""
    hi, lo = _hi_lo(blk)
    return np.concatenate([hi, hi, lo], axis=-2)


def _rhs_stack(cvecs):
    """cvecs [4, D] f64 -> [96, 512] fp16: blocks [c_hi; c_lo; c_hi] with
    rhs_blk[(k,jl), jl*64+d] = c_k[d] on the (k,jl) block diagonal."""
    hi, lo = _hi_lo(np.asarray(cvecs))
    out = np.zeros((K, RHS_C), np.float16)
    for blk, c in enumerate((hi, lo, hi)):
        for k in range(4):
            for jl in range(JW):
                out[blk * KB + k * JW + jl, jl * D:(jl + 1) * D] = c[k]
    return out


def kernel(pos, freq, W_da, b_da, W_dv, b_dv, W_ca, b_ca, W_cv, b_cv,
           W_bdl, b_bdl, edge_attention, edge_values,
           e2e_edge_attention, e2e_edge_values, src, dst):
    global _CACHED_NC, _LAST_RESULTS
    pos64 = np.asarray(pos, np.float64)
    freq64 = np.asarray(freq, np.float64)
    src_i = np.asarray(src)
    dst_i = np.asarray(dst)

    node_blk, e2e_blk, bdl_blk = _host_factors(pos64, freq64, src_i, dst_i)
    node_lhsT = _lhsT_stack(node_blk)       # [B,96,2048]
    e2e_lhsT = _lhsT_stack(e2e_blk)         # [B,96,8192]
    bdl_lhsT = _lhsT_stack(bdl_blk)         # [B,12,256]

    rhs_na = _rhs_stack([np.asarray(W_da, np.float64)[:, k] for k in range(3)]
                        + [np.asarray(b_da, np.float64)])
    rhs_nv = _rhs_stack([np.asarray(W_dv, np.float64)[:, k] for k in range(3)]
                        + [np.asarray(b_dv, np.float64)])

    def cos_coeffs(W, b):
        W = np.asarray(W, np.float64)
        b = np.asarray(b, np.float64)
        alpha = b + C0 * W[:, 0] - C2 * W[:, 2]
        beta = C1 * W[:, 1]
        gamma = 3.0 * C2 * W[:, 2]
        return [alpha, beta, gamma, np.zeros(D)]

    rhs_ea = _rhs_stack(cos_coeffs(W_ca, b_ca))
    rhs_ev = _rhs_stack(cos_coeffs(W_cv, b_cv))
    bhi, blo = _hi_lo(np.stack(
        [np.asarray(W_bdl, np.float64)[:, k] for k in range(3)]
        + [np.asarray(b_bdl, np.float64)]))  # [4, D]
    bdl_rhs = np.concatenate([bhi, blo, bhi])  # [12, D] fp16

    in_maps = []
    for b in range(B):
        pk = np.zeros((K, PACKW), np.float16)
        pk[:, OFF_NODE:OFF_NODE + NODE_LHST_C] = node_lhsT[b]
        pk[:, OFF_E2E:OFF_E2E + E2E_LHST_C] = e2e_lhsT[b]
        pk[:, OFF_RNA:OFF_RNA + RHS_C] = rhs_na
        pk[:, OFF_RNV:OFF_RNV + RHS_C] = rhs_nv
        pk[:, OFF_REA:OFF_REA + RHS_C] = rhs_ea
        pk[:, OFF_REV:OFF_REV + RHS_C] = rhs_ev
        pk[0:12, OFF_BDL:OFF_BDL + E] = bdl_lhsT[b]
        pk[0:12, OFF_BRHS:OFF_BRHS + D] = bdl_rhs
        in_maps.append({
            "ea": np.ascontiguousarray(
                np.asarray(edge_attention[b], np.float32).reshape(N, NODE_W)),
            "ev": np.ascontiguousarray(
                np.asarray(edge_values[b], np.float32).reshape(N, NODE_W)),
            "e2a": np.ascontiguousarray(
                np.asarray(e2e_edge_attention[b], np.float32).reshape(E, E2E_W)),
            "e2v": np.ascontiguousarray(
                np.asarray(e2e_edge_values[b], np.float32).reshape(E, E2E_W)),
            "pk": pk,
        })

    if _CACHED_NC is None:
        _CACHED_NC = _build_nc()
    res = run_bass_kernel_spmd(_CACHED_NC, in_maps, core_ids=list(range(B)))
    _LAST_RESULTS = res

    out0 = np.stack([res.results[b]["oa"].reshape(N, N, D) for b in range(B)])
    out1 = np.stack([res.results[b]["ov"].reshape(N, N, D) for b in range(B)])
    out2 = np.stack([res.results[b]["o2a"].reshape(E, E, D) for b in range(B)])
    out3 = np.stack([res.results[b]["o2v"].reshape(E, E, D) for b in range(B)])
    out4 = np.stack([res.results[b]["obdl"] for b in range(B)])
    return (out0.astype(np.float32), out1.astype(np.float32),
            out2.astype(np.float32), out3.astype(np.float32),
            out4.astype(np.float32))


# revision 14
# speedup vs baseline: 1.1764x; 1.1764x over previous
"""Trainium2 Bass kernel for nn_PositionEncoder (gnn_message_passing).

Strategy
--------
Data parallel over batch B=8: core b processes graph b (sharding_hint).

The reference reduces to four big elementwise streaming updates plus one
small output:

    out0 = edge_attention     + expand(node_s, c_na)     [128,128,64]
    out1 = edge_values        + expand(node_s, c_nv)     [128,128,64]
    out2 = e2e_edge_attention + expand(e2e_s,  c_ea)     [256,256,64]
    out3 = e2e_edge_values    + expand(e2e_s,  c_ev)     [256,256,64]
    out4 = expand(bdl_s, c_b)                            [256,64]

where every `expand` is a rank-4 broadcast product
    delta[i,j,d] = sum_k s_k[i,j] * c_k[d]
because:
  * dis_att/dis_val = Linear(rbf(dis)) with rbf giving 3 scalar fields
    r_k[i,j] (plus the bias as a 4th, constant field),
  * cos(arccos(clip(q))) == clip(q), so cbf is a per-element quadratic in
    x = clip(cos*m, -1, 1) and cos_att/cos_val reduce to
    u*alpha[d] + x*beta[d] + x^2*gamma[d]  (u = |m|).

The s_k fields are tiny ([N,N] / [E,E] scalars, ~2% of the bytes) and are
precomputed on host into a packed lhsT layout; the device streams the 40MB
per-core payload once, expanding the rank-4 products on the TensorEngine,
then one VectorEngine add per 2048-column PSUM quad, and streams out.

TensorEngine details: fp32 matmuls run at 1/4 rate, so operands are fp16
with double-fp16 compensation — each rank k is split into (hi, lo) halves
and the K dim packs three product blocks (s_hi*c_hi + s_hi*c_lo +
s_lo*c_hi), recovering ~fp32 accuracy while keeping fp16 throughput
(LDWEIGHTS cost depends on columns, matmul cost on N; K is nearly free).
rhs holds c_k[d] on a (k,jl) block diagonal so one K=96 matmul produces a
[128 rows, 8*64] PSUM block. This keeps the kernel at the HBM roofline
(memory-bound regime).
"""

import numpy as np

import concourse.bacc as bacc
import concourse.mybir as mybir
from concourse.tile import TileContext
from concourse.bass_utils import run_bass_kernel_spmd

# Problem shapes (hardcoded per task instructions).
B, N, E, D = 8, 128, 256, 64
CUTOFF = 5.0
P = 6  # envelope_exponent 5 -> p = 6
EA = -(P + 1) * (P + 2) / 2.0
EB = float(P * (P + 2))
EC = -P * (P + 1) / 2.0
C0 = 0.28209479177387814
C1 = 0.4886025119029199
C2 = 0.31539156525252005

JW = 8            # j's packed per matmul group
KB = 32           # 4 ranks x JW (one product block)
K = 3 * KB        # 96: [s_hi | s_hi | s_lo] x [c_hi | c_lo | c_hi]
NODE_G = N // JW          # 16 groups
E2E_G = E // JW           # 32 groups
NODE_W = N * D            # 8192 free cols per node tile
E2E_W = E * D             # 16384 free cols per e2e row-chunk

# Packed-constant tensor column layout: [K=96, PACKW] fp16
NODE_LHST_C = NODE_G * 128            # 2048
E2E_LHST_C = 2 * E2E_G * 128          # 8192
RHS_C = JW * D                        # 512
OFF_NODE = 0
OFF_E2E = OFF_NODE + NODE_LHST_C
OFF_RNA = OFF_E2E + E2E_LHST_C
OFF_RNV = OFF_RNA + RHS_C
OFF_REA = OFF_RNV + RHS_C
OFF_REV = OFF_REA + RHS_C
OFF_BDL = OFF_REV + RHS_C             # bdl lhsT: rows 0..11, E cols
OFF_BRHS = OFF_BDL + E                # bdl rhs: rows 0..11, D cols
PACKW = OFF_BRHS + D

QW = 2048          # PSUM quad width (4 banks)
GPQ = QW // RHS_C  # matmul groups per quad = 4
CW = 4096          # DMA chunk width (2 quads)

_CACHED_NC = None
_LAST_RESULTS = None


def _build_nc():
    nc = bacc.Bacc(None, target_bir_lowering=False)
    f32 = mybir.dt.float32
    f16 = mybir.dt.float16

    ea = nc.dram_tensor("ea", [N, NODE_W], f32, kind="ExternalInput")
    ev = nc.dram_tensor("ev", [N, NODE_W], f32, kind="ExternalInput")
    e2a = nc.dram_tensor("e2a", [E, E2E_W], f32, kind="ExternalInput")
    e2v = nc.dram_tensor("e2v", [E, E2E_W], f32, kind="ExternalInput")
    pk = nc.dram_tensor("pk", [K, PACKW], f16, kind="ExternalInput")

    oa = nc.dram_tensor("oa", [N, NODE_W], f32, kind="ExternalOutput")
    ov = nc.dram_tensor("ov", [N, NODE_W], f32, kind="ExternalOutput")
    o2a = nc.dram_tensor("o2a", [E, E2E_W], f32, kind="ExternalOutput")
    o2v = nc.dram_tensor("o2v", [E, E2E_W], f32, kind="ExternalOutput")
    obdl = nc.dram_tensor("obdl", [E, D], f32, kind="ExternalOutput")

    with TileContext(nc) as tc:
        with (
            tc.tile_pool(name="const", bufs=1) as cpool,
            tc.tile_pool(name="sbuf", bufs=10) as pool,
            tc.tile_pool(name="psum", bufs=2, space="PSUM") as psum_pool,
            tc.tile_pool(name="bdl", bufs=2) as bdl_pool,
        ):
            pk_t = cpool.tile([K, PACKW], f16)
            nc.sync.dma_start(out=pk_t[:], in_=pk[:])

            # bdl output: 2 chunks of 128 edges
            for c in range(2):
                ps = psum_pool.tile([128, D], f32, tag="ps")
                nc.tensor.matmul(
                    ps[:],
                    pk_t[0:12, OFF_BDL + c * 128:OFF_BDL + (c + 1) * 128],
                    pk_t[0:12, OFF_BRHS:OFF_BRHS + D],
                    start=True, stop=True,
                )
                st = bdl_pool.tile([128, D], f32)
                nc.scalar.copy(out=st[:], in_=ps[:])
                nc.scalar.dma_start(out=obdl[c * 128:(c + 1) * 128, :], in_=st[:])

            # (dram_in, dram_out, rhs column base, lhsT col base, row chunks,
            #  width, chunk width). Big e2e tensors first; node last with
            #  smaller chunks so the end-of-kernel drain chain is short.
            work = [
                (e2a, o2a, OFF_REA, OFF_E2E, 2, E2E_W, CW),
                (e2v, o2v, OFF_REV, OFF_E2E, 2, E2E_W, CW),
                (ea, oa, OFF_RNA, OFF_NODE, 1, NODE_W, QW),
                (ev, ov, OFF_RNV, OFF_NODE, 1, NODE_W, QW),
            ]
            for src, dst, rhs_off, lhs_off, nchunks, width, cw in work:
                rhs_ap = pk_t[:, rhs_off:rhs_off + RHS_C]
                for ic in range(nchunks):
                    r0 = ic * 128
                    # lhsT columns for this row-chunk start at lhs_off + ic*G*128
                    gbase = lhs_off + ic * (width // RHS_C) * 128
                    for cc in range(width // cw):
                        t = pool.tile([128, CW], f32, tag="t")
                        nc.sync.dma_start(
                            out=t[:, 0:cw],
                            in_=src[r0:r0 + 128, cc * cw:(cc + 1) * cw],
                        )
                        for half in range(cw // QW):
                            ps = psum_pool.tile([128, QW], f32, tag="ps")
                            for q in range(GPQ):
                                g = (cc * (cw // QW) + half) * GPQ + q
                                nc.tensor.matmul(
                                    ps[:, q * RHS_C:(q + 1) * RHS_C],
                                    pk_t[:, gbase + g * 128:gbase + (g + 1) * 128],
                                    rhs_ap,
                                    start=True, stop=True,
                                )
                            h0 = half * QW
                            nc.vector.tensor_add(
                                out=t[:, h0:h0 + QW],
                                in0=t[:, h0:h0 + QW],
                                in1=ps[:],
                            )
                        # stores on the ACT HWDGE ring so loads (sync
                        # ring) and stores drain concurrently
                        nc.scalar.dma_start(
                            out=dst[r0:r0 + 128, cc * cw:(cc + 1) * cw],
                            in_=t[:, 0:cw],
                        )
    nc.finalize()
    return nc


def _envelope_rbf(x, freq):
    """r_k = envelope(x) * sin(freq_k * x); freq [3]."""
    inv = 1.0 / x
    x4 = (x * x) * (x * x)
    xp0 = x4 * x  # x^5
    env = inv + EA * xp0 + EB * xp0 * x + EC * xp0 * x * x
    env = np.where(x < 1.0, env, 0.0)
    return env[..., None] * np.sin(freq * x[..., None])


def _hi_lo(a):
    """Split float64 array into fp16 hi + fp16 lo with a ~= hi + lo."""
    hi = a.astype(np.float16)
    lo = (a - hi.astype(np.float64)).astype(np.float16)
    return hi, lo


def _host_factors(pos, freq, src, dst):
    """Packed lhsT factor tensors for all batches (fp16 hi/lo blocks)."""
    # --- node pairwise rbf factors ---
    diff = pos[:, :, None, :] - pos[:, None, :, :]          # [B,N,N,3]
    dis = np.sqrt((diff * diff).sum(-1))                    # [B,N,N]
    x = (dis + 1e-6) / CUTOFF
    r = _envelope_rbf(x, freq)                              # [B,N,N,3]
    # block[b, k*8+jl, g*128+n1] = s_k[b, n1, g*8+jl]
    A = r.transpose(0, 3, 2, 1)                             # [B,3,n2,n1]
    A = A.reshape(B, 3, NODE_G, JW, N).transpose(0, 1, 3, 2, 4)
    A = A.reshape(B, 24, NODE_G * N)
    node_blk = np.concatenate(
        [A, np.ones((B, 8, NODE_G * N))], axis=1)           # [B,32,2048] f64

    # --- edge vectors ---
    vec = (np.take_along_axis(pos, dst[..., None], axis=1)
           - np.take_along_axis(pos, src[..., None], axis=1))  # [B,E,3]
    vn = np.sqrt((vec * vec).sum(-1))                       # [B,E]
    rb = _envelope_rbf(vn / CUTOFF, freq)                   # [B,E,3]
    bdl_blk = np.concatenate(
        [rb.transpose(0, 2, 1), np.ones((B, 1, E))], axis=1)  # [B,4,E] f64

    # --- e2e cos factors ---
    g = np.einsum('bid,bjd->bij', vec, vec)                 # [B,E,E]
    cos = g / (vn[:, None, :] + 1e-6) / (vn[:, :, None] + 1e-6)
    si, sj = src[:, :, None], src[:, None, :]
    di, dj = dst[:, :, None], dst[:, None, :]
    m = np.zeros((B, E, E), np.int64)
    m = np.where((si == sj) & (di != dj), 1, m)
    m = np.where((di == dj) & (si != sj), 1, m)
    m = np.where(si == dj, -1, m)
    m = np.where(di == sj, -1, m)
    u = np.abs(m).astype(np.float64)
    xh = np.clip(cos * m, -1.0, 1.0)
    s = np.stack([u, xh, xh * xh, np.zeros_like(u)], axis=1)  # [B,4,i,j]
    # block[b, k*8+jl, (ic*32+g)*128+il] = s_k[b, ic*128+il, g*8+jl];
    # all s_k are symmetric in (i,j) -> use [b, k, j, i] directly.
    S = s.transpose(0, 1, 3, 2)
    S = S.reshape(B, 4, E2E_G, JW, E).transpose(0, 1, 3, 2, 4)  # [B,4,jl,g,i]
    S = S.reshape(B, 4, JW, E2E_G, 2, 128).transpose(0, 1, 2, 4, 3, 5)
    e2e_blk = S.reshape(B, KB, E2E_LHST_C)                  # f64
    return node_blk, e2e_blk, bdl_blk


def _lhsT_stack(blk):
    """[.., KB, C] f64 -> [.., 3*KB, C] fp16 as [hi; hi; lo]."""
    hi, lo = _hi_lo(blk)
    return np.concatenate([hi, hi, lo], axis=-2)


def _rhs_stack(cvecs):
    """cvecs [4, D] f64 -> [96, 512] fp16: blocks [c_hi; c_lo; c_hi] with
    rhs_blk[(k,jl), jl*64+d] = c_k[d] on the (k,jl) block diagonal."""
    hi, lo = _hi_lo(np.asarray(cvecs))
    out = np.zeros((K, RHS_C), np.float16)
    for blk, c in enumerate((hi, lo, hi)):
        for k in range(4):
            for jl in range(JW):
                out[blk * KB + k * JW + jl, jl * D:(jl + 1) * D] = c[k]
    return out


def kernel(pos, freq, W_da, b_da, W_dv, b_dv, W_ca, b_ca, W_cv, b_cv,
           W_bdl, b_bdl, edge_attention, edge_values,
           e2e_edge_attention, e2e_edge_values, src, dst):
    global _CACHED_NC, _LAST_RESULTS
    pos64 = np.asarray(pos, np.float64)
    freq64 = np.asarray(freq, np.float64)
    src_i = np.asarray(src)
    dst_i = np.asarray(dst)

    node_blk, e2e_blk, bdl_blk = _host_factors(pos64, freq64, src_i, dst_i)
    node_lhsT = _lhsT_stack(node_blk)       # [B,96,2048]
    e2e_lhsT = _lhsT_stack(e2e_blk)         # [B,96,8192]
    bdl_lhsT = _lhsT_stack(bdl_blk)         # [B,12,256]

    rhs_na = _rhs_stack([np.asarray(W_da, np.float64)[:, k] for k in range(3)]
                        + [np.asarray(b_da, np.float64)])
    rhs_nv = _rhs_stack([np.asarray(W_dv, np.float64)[:, k] for k in range(3)]
                        + [np.asarray(b_dv, np.float64)])

    def cos_coeffs(W, b):
        W = np.asarray(W, np.float64)
        b = np.asarray(b, np.float64)
        alpha = b + C0 * W[:, 0] - C2 * W[:, 2]
        beta = C1 * W[:, 1]
        gamma = 3.0 * C2 * W[:, 2]
        return [alpha, beta, gamma, np.zeros(D)]

    rhs_ea = _rhs_stack(cos_coeffs(W_ca, b_ca))
    rhs_ev = _rhs_stack(cos_coeffs(W_cv, b_cv))
    bhi, blo = _hi_lo(np.stack(
        [np.asarray(W_bdl, np.float64)[:, k] for k in range(3)]
        + [np.asarray(b_bdl, np.float64)]))  # [4, D]
    bdl_rhs = np.concatenate([bhi, blo, bhi])  # [12, D] fp16

    in_maps = []
    for b in range(B):
        pk = np.zeros((K, PACKW), np.float16)
        pk[:, OFF_NODE:OFF_NODE + NODE_LHST_C] = node_lhsT[b]
        pk[:, OFF_E2E:OFF_E2E + E2E_LHST_C] = e2e_lhsT[b]
        pk[:, OFF_RNA:OFF_RNA + RHS_C] = rhs_na
        pk[:, OFF_RNV:OFF_RNV + RHS_C] = rhs_nv
        pk[:, OFF_REA:OFF_REA + RHS_C] = rhs_ea
        pk[:, OFF_REV:OFF_REV + RHS_C] = rhs_ev
        pk[0:12, OFF_BDL:OFF_BDL + E] = bdl_lhsT[b]
        pk[0:12, OFF_BRHS:OFF_BRHS + D] = bdl_rhs
        in_maps.append({
            "ea": np.ascontiguousarray(
                np.asarray(edge_attention[b], np.float32).reshape(N, NODE_W)),
            "ev": np.ascontiguousarray(
                np.asarray(edge_values[b], np.float32).reshape(N, NODE_W)),
            "e2a": np.ascontiguousarray(
                np.asarray(e2e_edge_attention[b], np.float32).reshape(E, E2E_W)),
            "e2v": np.ascontiguousarray(
                np.asarray(e2e_edge_values[b], np.float32).reshape(E, E2E_W)),
            "pk": pk,
        })

    if _CACHED_NC is None:
        _CACHED_NC = _build_nc()
    res = run_bass_kernel_spmd(_CACHED_NC, in_maps, core_ids=list(range(B)))
    _LAST_RESULTS = res

    out0 = np.stack([res.results[b]["oa"].reshape(N, N, D) for b in range(B)])
    out1 = np.stack([res.results[b]["ov"].reshape(N, N, D) for b in range(B)])
    out2 = np.stack([res.results[b]["o2a"].reshape(E, E, D) for b in range(B)])
    out3 = np.stack([res.results[b]["o2v"].reshape(E, E, D) for b in range(B)])
    out4 = np.stack([res.results[b]["obdl"] for b in range(B)])
    return (out0.astype(np.float32), out1.astype(np.float32),
            out2.astype(np.float32), out3.astype(np.float32),
            out4.astype(np.float32))
